# revision 17
# baseline (speedup 1.0000x reference)
"""CodebookLoRASTELinear forward on 8 Trainium2 NeuronCores.

out = x @ (W_q + D)^T
  D   = (lora_B @ lora_A) * (alpha/rank)
  cb  = codebook / max|codebook|
  S   = exp(scale_log)                     (per [o, i//128] group)
  q   = cb[searchsorted(midpoints(cb), (W+D)/S)]
      == cm + sum_k (d_k/2) * sign((W+D)/S - t_k),  cm = cb0 + sum_k d_k/2
  W_eff = q*S + D

Column-parallel sharding: W / scale / lora_B rows (out_features) are split
across the 8 cores; x and lora_A are replicated; per-core outputs are
concatenated on the host (no collectives).

The host pre-transposes x to x^T [I, M] in bf16, so phase C is pure bf16
matmul (1 cyc/row, no PE transposes of x) and x HBM traffic is halved.

Phase B computes, per pair of i-groups:
  PE:  Dc = D + cm*S (augmented-rank lora matmul, f32r 256-wide moving)
       u' = w + Dc   (same matmul + identity-matmul accumulate of w)
  DVE: v' = u' * invS_rep   (invS replicated along i, streamed from host,
       so thresholds are CONSTANT and every op is a full-tile op)
  ACT: s_k = Sign(v' - (cm + t_k)), k=1..3, output bf16 (+-1 exact)
  DVE: staircase in bf16 (tensor_scalar 4x / tensor_tensor 2x DVE modes):
       w_k = s_k * d_k/2;  a = w1+w2+w3 (= q - cm, exact in bf16);
       m = a * S_rep(bf16);  w_eff = m + Dc  -> bf16
  PE:  transpose w_eff tiles into folded W_eff^T [i%128, g, o]
(one tensor_tensor add is placed on GPSIMD per pair to offload DVE).

Phase C streams x^T chunks and accumulates out[m(128), o(512)] over the 32
i-groups with bf16 matmuls (moving operand 512 wide, 1 cyc/row).
"""

import numpy as np
import sys

for _p in ("/opt/trn_rl_repo",):
    if _p not in sys.path:
        sys.path.insert(0, _p)

import ml_dtypes  # noqa: E402
import concourse.mybir as mybir  # noqa: E402
import concourse.tile as tile  # noqa: E402
from concourse import bacc  # noqa: E402
from concourse.bass_utils import run_bass_kernel_spmd  # noqa: E402
from concourse.masks import make_identity  # noqa: E402
from contextlib import ExitStack  # noqa: E402

N_CORES = 8
M = 8192  # 4 * 2048 tokens
I = 4096  # in_features
O = 4096  # out_features
GROUP = 128
NG = I // GROUP  # 32 groups along i
RANK = 64
KAUG = RANK + NG  # 96: lora rank + one row per group for cm*S
ALPHA_OVER_RANK = 32.0 / 64.0
OS = O // N_CORES  # 512 out features per core
NOB = OS // 128  # 4 output row blocks per core
MC = 256  # phase C m-chunk
NMC = M // MC  # 32 m chunks

F32 = mybir.dt.float32
F32R = mybir.dt.float32r
BF16 = mybir.dt.bfloat16
BF16_NP = ml_dtypes.bfloat16

_cache = {}


def _build_program(cb0, tk, dk, reps=1):
    """cb0: smallest normalized codebook entry; tk: 3 bucket thresholds;
    dk: 3 successive codebook differences. All host floats baked in."""
    nc = bacc.Bacc("TRN2", target_bir_lowering=False, debug=False)

    xt_d = nc.dram_tensor("xt", [I, M], BF16, kind="ExternalInput").ap()
    # w split into two bf16 terms (w ~= wh + wl, residual ~1.5e-5 relative):
    # the identity-matmul accumulation of w into u' is then exact in bf16,
    # avoiding f32r's ~2^-11 rounding of w which flips quantization buckets
    wh_d = nc.dram_tensor("wh", [OS, I], BF16, kind="ExternalInput").ap()
    wl_d = nc.dram_tensor("wl", [OS, I], BF16, kind="ExternalInput").ap()
    isr_d = nc.dram_tensor("isr", [OS, I], F32, kind="ExternalInput").ap()
    sr_d = nc.dram_tensor("sr", [OS, I], BF16, kind="ExternalInput").ap()
    la_d = nc.dram_tensor("la", [KAUG, I], F32R, kind="ExternalInput").ap()
    lbt_d = nc.dram_tensor("lbt", [KAUG, OS], F32R, kind="ExternalInput").ap()
    out_d = nc.dram_tensor("out", [M, OS], F32, kind="ExternalOutput").ap()

    cm = float(cb0) + float(sum(dk)) * 0.5

    with tile.TileContext(nc) as tc, ExitStack() as ctx:
        singles = ctx.enter_context(tc.tile_pool(name="singles", bufs=1))

        ident = singles.tile([128, 128], F32)
        make_identity(nc, ident)
        identb = singles.tile([128, 128], BF16)
        nc.vector.tensor_copy(identb, ident)
        identr = singles.tile([128, 128], F32R)
        nc.vector.tensor_copy(identr, ident)

        # constant sign biases -(cm + t_k), one per threshold
        bias = []
        for k in range(3):
            b = singles.tile([128, 1], F32, tag=f"bias{k}")
            nc.vector.memset(b, -(cm + float(tk[k])))
            bias.append(b)

        lbt_sb = singles.tile([KAUG, OS], F32R)
        nc.sync.dma_start(out=lbt_sb, in_=lbt_d)
        lbt_r = lbt_sb
        # la in 4 independently-loaded tiles so the first lora matmul only
        # waits on the first quarter
        la_parts, la_rparts = [], []
        for j in range(4):
            lp = singles.tile([KAUG, I // 4], F32R, tag=f"la{j}")
            nc.sync.dma_start(out=lp, in_=la_d[:, j * (I // 4) : (j + 1) * (I // 4)])
            la_parts.append(lp)
            la_rparts.append(lp)

        # persistent effective transposed weight, folded [i%128, g, o], bf16
        weff = singles.tile([128, NG, OS], BF16)

        if reps > 1:
            ctx.enter_context(tc.For_i(0, reps, 1))

        # ---- phase B (g-pair-major) ----------------------------------------
        wload = ctx.enter_context(tc.tile_pool(name="wload", bufs=2))
        ispool = ctx.enter_context(tc.tile_pool(name="ispool", bufs=2))
        srpool = ctx.enter_context(tc.tile_pool(name="srpool", bufs=2))
        vpool = ctx.enter_context(tc.tile_pool(name="vpool", bufs=2))
        spool = ctx.enter_context(tc.tile_pool(name="spool", bufs=2))
        cpool = ctx.enter_context(tc.tile_pool(name="cpool", bufs=2))
        wqpool = ctx.enter_context(tc.tile_pool(name="wq", bufs=2))
        psumA = ctx.enter_context(tc.tile_pool(name="psumA", bufs=2, space="PSUM"))
        psumB = ctx.enter_context(tc.tile_pool(name="psumB", bufs=1, space="PSUM"))
        # transpose staging shares PSUM banks with phase C's output pool
        psumX = ctx.enter_context(tc.tile_pool(name="psumX", bufs=2, space="PSUM"))

        rearr = "(ob p) (gg i) -> p ob gg i"
        NGP = NG // 2

        def b_front(gp):
            """DMA loads + PE matmuls for gpair gp; returns live tiles."""
            g0 = 2 * gp
            gsl = slice(g0 * 128, (g0 + 2) * 128)
            wh_sb = wload.tile([128, NOB, 2, 128], BF16, tag="wh")
            nc.sync.dma_start(out=wh_sb, in_=wh_d[:, gsl].rearrange(rearr, p=128, gg=2))
            wl_sb = wload.tile([128, NOB, 2, 128], BF16, tag="wl")
            nc.sync.dma_start(out=wl_sb, in_=wl_d[:, gsl].rearrange(rearr, p=128, gg=2))
            is_sb = ispool.tile([128, NOB, 2, 128], F32, tag="is")
            nc.sync.dma_start(out=is_sb, in_=isr_d[:, gsl].rearrange(rearr, p=128, gg=2))
            sr_sb = srpool.tile([128, NOB, 2, 128], BF16, tag="sr")
            nc.sync.dma_start(out=sr_sb, in_=sr_d[:, gsl].rearrange(rearr, p=128, gg=2))

            la_part = la_rparts[gp // 4]
            lsl = slice(g0 * 128 - (gp // 4) * (I // 4),
                        (g0 + 2) * 128 - (gp // 4) * (I // 4))
            # Dc = D + cm*S via augmented-rank f32r matmul (256-wide moving)
            d_a = psumA.tile([128, NOB, 2, 128], F32, tag="da")
            for ob in range(NOB):
                nc.tensor.matmul(
                    d_a[:, ob, :, :],
                    lhsT=lbt_r[:, ob * 128 : (ob + 1) * 128],
                    rhs=la_part[:, lsl],
                    start=True,
                    stop=True,
                )
            # u' = w + Dc: same matmul plus identity-matmul accumulate of w
            d_b = psumB.tile([128, NOB, 2, 128], F32, tag="db")
            for ob in range(NOB):
                nc.tensor.matmul(
                    d_b[:, ob, :, :],
                    lhsT=lbt_r[:, ob * 128 : (ob + 1) * 128],
                    rhs=la_part[:, lsl],
                    start=True,
                    stop=False,
                )
                nc.tensor.matmul(
                    d_b[:, ob, :, :],
                    lhsT=identb,
                    rhs=wh_sb[:, ob, :, :],
                    start=False,
                    stop=False,
                )
                nc.tensor.matmul(
                    d_b[:, ob, :, :],
                    lhsT=identb,
                    rhs=wl_sb[:, ob, :, :],
                    start=False,
                    stop=True,
                )
            return is_sb, sr_sb, d_a, d_b

        def b_chain(gp, tiles):
            """Elementwise quantization chain for gpair gp (DVE/ACT/Pool)."""
            is_sb, sr_sb, d_a, d_b = tiles
            # v' = u' * invS  (thresholds constant in this domain)
            v = vpool.tile([128, NOB, 2, 128], F32, tag="v")
            nc.vector.tensor_mul(v, d_b, is_sb)

            # s_k = Sign(v' - (cm + t_k)) in bf16 (+-1 exact)
            s1 = spool.tile([128, NOB, 2, 128], BF16, tag="s1")
            nc.scalar.sign(s1, v, bias=bias[0])
            s2 = spool.tile([128, NOB, 2, 128], BF16, tag="s2")
            nc.scalar.sign(s2, v, bias=bias[1])
            s3 = spool.tile([128, NOB, 2, 128], BF16, tag="s3")
            nc.scalar.sign(s3, v, bias=bias[2])

            # bf16 staircase: a = sum_k (d_k/2) s_k  == q - cm (exact)
            w1 = cpool.tile([128, NOB, 2, 128], BF16, tag="w1")
            nc.vector.tensor_scalar_mul(w1, s1, float(dk[0]) * 0.5)
            w2 = cpool.tile([128, NOB, 2, 128], BF16, tag="w2")
            nc.vector.tensor_scalar_mul(w2, s2, float(dk[1]) * 0.5)
            w3 = cpool.tile([128, NOB, 2, 128], BF16, tag="w3")
            nc.vector.tensor_scalar_mul(w3, s3, float(dk[2]) * 0.5)
            a12 = cpool.tile([128, NOB, 2, 128], BF16, tag="a12")
            nc.gpsimd.tensor_add(a12, w1, w2)
            a = cpool.tile([128, NOB, 2, 128], BF16, tag="a")
            nc.vector.tensor_add(a, a12, w3)
            # m = (q - cm) * S
            m = cpool.tile([128, NOB, 2, 128], BF16, tag="m")
            nc.vector.tensor_mul(m, a, sr_sb)
            # w_eff = m + Dc  (Dc = D + cm*S; GPSIMD cannot read PSUM)
            wq = wqpool.tile([128, NOB, 2, 128], BF16, tag="wq")
            nc.vector.tensor_add(wq, m, d_a)
            return wq

        def b_back(gp, wq):
            """Transpose + copy W_eff^T rows for gpair gp."""
            g0 = 2 * gp
            pt = psumX.tile([128, 2, NOB, 128], BF16, tag="ps")
            for gg in range(2):
                for ob in range(NOB):
                    nc.tensor.transpose(pt[:, gg, ob, :], wq[:, ob, gg, :], identb)
            # one copy lands both group rows of W_eff^T; alternate engines
            if gp % 2 == 0:
                nc.scalar.copy(weff[:, g0 : g0 + 2, :], pt)
            else:
                nc.vector.tensor_copy(weff[:, g0 : g0 + 2, :], pt)

        # ---- phase C pools + x prefetch (issued before phase B so the DMA
        # engine fills the first chunks while quantization runs) -------------
        xpool = ctx.enter_context(tc.tile_pool(name="xpool", bufs=2))
        opool = ctx.enter_context(tc.tile_pool(name="opool", bufs=2))
        xt_r = xt_d.rearrange("(g p) m -> p g m", p=128)

        def x_load(t):
            msl = slice(t * MC, (t + 1) * MC)
            xsb = xpool.tile([128, NG, MC], BF16, tag="x")
            nc.sync.dma_start(out=xsb, in_=xt_r[:, :, msl])
            return xsb

        xq = []

        # software pipeline: issue gpair gp+1's PE matmuls before gpair gp's
        # transposes so the in-order PE queue never stalls on the DVE chain
        tiles = b_front(0)
        wq_prev = None
        for gp in range(NGP):
            if wq_prev is not None:
                b_back(gp - 1, wq_prev)
            wq_prev = b_chain(gp, tiles)
            if gp + 1 < NGP:
                tiles = b_front(gp + 1)
            if gp == NGP - 2:
                # prefetch the first x chunks once phase B DMA traffic wanes
                xq = [x_load(0), x_load(1)]
        b_back(NGP - 1, wq_prev)

        # ---- phase C: stream x^T chunks, accumulate out tiles ---------------
        for t in range(NMC):
            msl = slice(t * MC, (t + 1) * MC)
            xsb = xq.pop(0)
            o_sb = opool.tile([128, MC // 128, OS], F32, tag="o")
            for j in range(MC // 128):
                p_out = psumX.tile([128, OS], F32, tag="ps")
                for g in range(NG):
                    nc.tensor.matmul(
                        p_out,
                        lhsT=xsb[:, g, j * 128 : (j + 1) * 128],
                        rhs=weff[:, g, :],
                        start=(g == 0),
                        stop=(g == NG - 1),
                    )
                nc.scalar.copy(o_sb[:, j, :], p_out)
            nc.sync.dma_start(
                out=out_d[msl, :].rearrange("(j p) o -> p j o", p=128),
                in_=o_sb,
            )
            if t + 2 < NMC:
                xq.append(x_load(t + 2))

    nc.compile()
    return nc


def _get_program(cb0, tk, dk, reps=1):
    key = (round(float(cb0), 9), tuple(round(float(t), 9) for t in tk),
           tuple(round(float(d), 9) for d in dk), reps)
    if key not in _cache:
        _cache[key] = _build_program(cb0, tk, dk, reps)
    return _cache[key]


def _make_in_maps(x, weight, scale_log, codebook, lora_A, lora_B):
    cb = np.asarray(codebook, dtype=np.float64)
    cb = cb / max(float(np.max(np.abs(cb))), 1e-8)
    dk = np.diff(cb)
    cm = float(cb[0]) + float(np.sum(dk)) * 0.5

    xb = np.ascontiguousarray(x.reshape(M, I), dtype=np.float32).astype(BF16_NP)
    xt = np.ascontiguousarray(xb.T)

    sl64 = np.exp(np.asarray(scale_log, dtype=np.float64).reshape(O, NG))
    s_full = sl64.astype(np.float32)            # S, f32 (matches on-host exp)
    is_full = (1.0 / sl64).astype(np.float32)   # 1/S, f32
    s_rep = np.repeat(s_full.astype(BF16_NP), GROUP, axis=1)   # [O, I] bf16
    is_rep = np.repeat(is_full, GROUP, axis=1)                 # [O, I] f32

    # indicator rows for the cm*S augmentation
    ind = np.zeros((NG, I), dtype=np.float32)
    for g in range(NG):
        ind[g, g * GROUP : (g + 1) * GROUP] = 1.0
    la_aug = np.concatenate(
        [np.ascontiguousarray(lora_A, dtype=np.float32), ind], axis=0)

    in_maps = []
    for c in range(N_CORES):
        sl = slice(c * OS, (c + 1) * OS)
        lbt = lora_B[sl].T.astype(np.float32) * ALPHA_OVER_RANK  # [64, OS]
        lbt_aug = np.concatenate([lbt, cm * s_full[sl].T], axis=0)  # [96, OS]
        wc = np.ascontiguousarray(weight[sl], dtype=np.float32)
        wh = wc.astype(BF16_NP)
        wl = (wc - wh.astype(np.float32)).astype(BF16_NP)
        in_maps.append({
            "xt": xt,
            "wh": wh,
            "wl": wl,
            "isr": np.ascontiguousarray(is_rep[sl]),
            "sr": np.ascontiguousarray(s_rep[sl]),
            "la": la_aug,
            "lbt": np.ascontiguousarray(lbt_aug),
        })
    return in_maps


def kernel(x, weight, scale_log, codebook, lora_A, lora_B):
    cb = np.asarray(codebook, dtype=np.float64)
    cb = cb / max(float(np.max(np.abs(cb))), 1e-8)
    tk = (cb[:-1] + cb[1:]) * 0.5
    dk = np.diff(cb)

    nc = _get_program(float(cb[0]), [float(v) for v in tk], [float(v) for v in dk])
    in_maps = _make_in_maps(x, weight, scale_log, codebook, lora_A, lora_B)

    res = run_bass_kernel_spmd(nc, in_maps, core_ids=list(range(N_CORES))).results
    out = np.concatenate([res[c]["out"] for c in range(N_CORES)], axis=1)
    return out.reshape(x.shape[0], x.shape[1], O)


# revision 22
# speedup vs baseline: 1.0309x; 1.0309x over previous
"""CodebookLoRASTELinear forward on 8 Trainium2 NeuronCores.

out = x @ (W_q + D)^T
  D   = (lora_B @ lora_A) * (alpha/rank)
  cb  = codebook / max|codebook|
  S   = exp(scale_log)                     (per [o, i//128] group)
  q   = cb[searchsorted(midpoints(cb), (W+D)/S)]
      == cm + sum_k (d_k/2) * sign((W+D)/S - t_k),  cm = cb0 + sum_k d_k/2
  W_eff = q*S + D

Column-parallel sharding: W / scale / lora_B rows (out_features) are split
across the 8 cores; x and lora_A are replicated; per-core outputs are
concatenated on the host (no collectives).

The host pre-transposes x to x^T [I, M] in bf16, so phase C is pure bf16
matmul (1 cyc/row, no PE transposes of x) and x HBM traffic is halved.

Phase B (quantization) runs entirely in the TRANSPOSED [i, o] domain, so
its output lands directly in the folded W_eff^T layout (no PE transposes,
no psum->weff copies). Per pair of i-groups (tiles [128 i%128, 2 gg, 512 o]):
  PE:  DcT  = (D + cm*S)^T   augmented-rank lora matmul, f32r 512-wide
       u'T  = w^T + DcT      same matmul + two bf16 identity-matmuls of the
                             bf16-split w^T (= wh + wl, residual ~1.5e-5:
                             exact in bf16, avoids f32r's 2^-11 rounding of
                             w which would flip quantization buckets)
  DVE: v'T  = u'T * invS^T_rep  (invS replicated along i, streamed from the
       host, so compare thresholds are CONSTANT -> full-tile ops)
  ACT: s_k = Sign(v'T - (cm + t_k)), k=1..3, bf16 (+-1 exact)
  PE:  qT = sum_k (d_k/2 * I) @ s_k   three scaled-identity bf16 matmuls
       accumulating the staircase in PSUM (exact: half-step sums of the
       normalized codebook are representable in bf16)
  DVE: m = qT * S^T_rep(bf16);  w_effT = m + DcT -> weff slice (bf16)

Phase C streams x^T chunks and accumulates out[m(128), o(512)] over the 32
i-groups with bf16 matmuls (moving operand 512 wide, 1 cyc/row).
"""

import numpy as np
import sys

for _p in ("/opt/trn_rl_repo",):
    if _p not in sys.path:
        sys.path.insert(0, _p)

import ml_dtypes  # noqa: E402
import concourse.mybir as mybir  # noqa: E402
import concourse.tile as tile  # noqa: E402
from concourse import bacc  # noqa: E402
from concourse.bass_utils import run_bass_kernel_spmd  # noqa: E402
from concourse.masks import make_identity  # noqa: E402
from contextlib import ExitStack  # noqa: E402

N_CORES = 8
M = 8192  # 4 * 2048 tokens
I = 4096  # in_features
O = 4096  # out_features
GROUP = 128
NG = I // GROUP  # 32 groups along i
RANK = 64
KAUG = RANK + NG  # 96: lora rank + one row per group for cm*S
ALPHA_OVER_RANK = 32.0 / 64.0
OS = O // N_CORES  # 512 out features per core
NOB = OS // 128  # 4 output row blocks per core
MC = 256  # phase C m-chunk
NMC = M // MC  # 32 m chunks

F32 = mybir.dt.float32
F32R = mybir.dt.float32r
BF16 = mybir.dt.bfloat16
BF16_NP = ml_dtypes.bfloat16

_cache = {}


def _build_program(cb0, tk, dk, reps=1):
    """cb0: smallest normalized codebook entry; tk: 3 bucket thresholds;
    dk: 3 successive codebook differences. All host floats baked in."""
    nc = bacc.Bacc("TRN2", target_bir_lowering=False, debug=False)

    xt_d = nc.dram_tensor("xt", [I, M], BF16, kind="ExternalInput").ap()
    wht_d = nc.dram_tensor("wht", [I, OS], BF16, kind="ExternalInput").ap()
    wlt_d = nc.dram_tensor("wlt", [I, OS], BF16, kind="ExternalInput").ap()
    ist_d = nc.dram_tensor("ist", [I, OS], F32, kind="ExternalInput").ap()
    srt_d = nc.dram_tensor("srt", [I, OS], BF16, kind="ExternalInput").ap()
    la_d = nc.dram_tensor("la", [KAUG, I], F32R, kind="ExternalInput").ap()
    lbt_d = nc.dram_tensor("lbt", [KAUG, OS], F32R, kind="ExternalInput").ap()
    out_d = nc.dram_tensor("out", [M, OS], F32, kind="ExternalOutput").ap()

    cm = float(cb0) + float(sum(dk)) * 0.5

    with tile.TileContext(nc) as tc, ExitStack() as ctx:
        singles = ctx.enter_context(tc.tile_pool(name="singles", bufs=1))

        ident = singles.tile([128, 128], F32)
        make_identity(nc, ident)
        identb = singles.tile([128, 128], BF16)
        nc.vector.tensor_copy(identb, ident)
        # scaled identities (d_k/2) * I for the staircase matmuls
        sident = []
        for k in range(3):
            sf = singles.tile([128, 128], F32, tag=f"sidf{k}")
            nc.vector.tensor_scalar_mul(sf, ident, float(dk[k]) * 0.5)
            sb = singles.tile([128, 128], BF16, tag=f"sidb{k}")
            nc.vector.tensor_copy(sb, sf)
            sident.append(sb)

        # constant sign biases -(cm + t_k), one per threshold
        bias = []
        for k in range(3):
            b = singles.tile([128, 1], F32, tag=f"bias{k}")
            nc.vector.memset(b, -(cm + float(tk[k])))
            bias.append(b)

        lbt_sb = singles.tile([KAUG, OS], F32R)
        nc.sync.dma_start(out=lbt_sb, in_=lbt_d)
        # la in 4 independently-loaded tiles so the first lora matmul only
        # waits on the first quarter
        la_parts = []
        for j in range(4):
            lp = singles.tile([KAUG, I // 4], F32R, tag=f"la{j}")
            nc.sync.dma_start(out=lp, in_=la_d[:, j * (I // 4) : (j + 1) * (I // 4)])
            la_parts.append(lp)

        # persistent effective transposed weight, folded [i%128, g, o], bf16
        weff = singles.tile([128, NG, OS], BF16)

        if reps > 1:
            ctx.enter_context(tc.For_i(0, reps, 1))

        # ---- phase B (g-pair-major, transposed domain) ----------------------
        whpool = ctx.enter_context(tc.tile_pool(name="whpool", bufs=2))
        ispool = ctx.enter_context(tc.tile_pool(name="ispool", bufs=2))
        srpool = ctx.enter_context(tc.tile_pool(name="srpool", bufs=2))
        vpool = ctx.enter_context(tc.tile_pool(name="vpool", bufs=2))
        spool = ctx.enter_context(tc.tile_pool(name="spool", bufs=3))
        mpool = ctx.enter_context(tc.tile_pool(name="mpool", bufs=3))
        psumA = ctx.enter_context(tc.tile_pool(name="psumA", bufs=2, space="PSUM"))
        psumB = ctx.enter_context(tc.tile_pool(name="psumB", bufs=1, space="PSUM"))
        # staircase accumulator shares its PSUM ring with phase C's p_out
        psumX = ctx.enter_context(tc.tile_pool(name="psumX", bufs=2, space="PSUM"))

        rearr = "(gg p) o -> p gg o"
        NGP = NG // 2

        def b_front(gp):
            """DMA loads + lora/identity PE matmuls for gpair gp."""
            g0 = 2 * gp
            rsl = slice(g0 * 128, (g0 + 2) * 128)
            wh_sb = whpool.tile([128, 2, OS], BF16, tag="wh")
            nc.sync.dma_start(out=wh_sb, in_=wht_d[rsl].rearrange(rearr, p=128))
            wl_sb = whpool.tile([128, 2, OS], BF16, tag="wl")
            nc.sync.dma_start(out=wl_sb, in_=wlt_d[rsl].rearrange(rearr, p=128))
            is_sb = ispool.tile([128, 2, OS], F32, tag="is")
            nc.sync.dma_start(out=is_sb, in_=ist_d[rsl].rearrange(rearr, p=128))
            sr_sb = srpool.tile([128, 2, OS], BF16, tag="sr")
            nc.sync.dma_start(out=sr_sb, in_=srt_d[rsl].rearrange(rearr, p=128))

            la_part = la_parts[gp // 4]
            # DcT = (D + cm*S)^T via augmented-rank f32r matmul (512 moving)
            d_a = psumA.tile([128, 2, OS], F32, tag="da")
            d_b = psumB.tile([128, 2, OS], F32, tag="db")
            for gg in range(2):
                g = g0 + gg
                lsl = slice(g * 128 - (gp // 4) * (I // 4),
                            (g + 1) * 128 - (gp // 4) * (I // 4))
                nc.tensor.matmul(
                    d_a[:, gg, :], lhsT=la_part[:, lsl], rhs=lbt_sb,
                    start=True, stop=True,
                )
                # u'^T = w^T + DcT: same matmul + exact bf16-split w adds
                nc.tensor.matmul(
                    d_b[:, gg, :], lhsT=la_part[:, lsl], rhs=lbt_sb,
                    start=True, stop=False,
                )
                nc.tensor.matmul(
                    d_b[:, gg, :], lhsT=identb, rhs=wh_sb[:, gg, :],
                    start=False, stop=False,
                )
                nc.tensor.matmul(
                    d_b[:, gg, :], lhsT=identb, rhs=wl_sb[:, gg, :],
                    start=False, stop=True,
                )
            return is_sb, sr_sb, d_a, d_b

        def b_chain(gp, tiles):
            """Signs + staircase + rescale for gpair gp; writes weff."""
            g0 = 2 * gp
            is_sb, sr_sb, d_a, d_b = tiles
            # v'^T = u'^T * invS^T (thresholds constant in this domain)
            v = vpool.tile([128, 2, OS], F32, tag="v")
            nc.vector.tensor_mul(v, d_b, is_sb)

            # s_k = Sign(v' - (cm + t_k)) in bf16 (+-1 exact)
            ss = []
            for k in range(3):
                s = spool.tile([128, 2, OS], BF16, tag=f"s{k}")
                nc.scalar.sign(s, v, bias=bias[k])
                ss.append(s)

            # staircase on the PE: qT[gg] = sum_k (d_k/2 I) @ s_k[gg] (exact)
            for gg in range(2):
                q_ps = psumX.tile([128, OS], F32, tag="ps")
                for k in range(3):
                    nc.tensor.matmul(
                        q_ps, lhsT=sident[k], rhs=ss[k][:, gg, :],
                        start=(k == 0), stop=(k == 2),
                    )
                # m = (q - cm)*S ; w_eff^T = m + DcT, straight into weff
                m = mpool.tile([128, OS], BF16, tag="m")
                nc.vector.tensor_mul(m, q_ps, sr_sb[:, gg, :])
                nc.vector.tensor_add(weff[:, g0 + gg, :], m, d_a[:, gg, :])

        # ---- phase C pools + x loader ---------------------------------------
        xpool = ctx.enter_context(tc.tile_pool(name="xpool", bufs=2))
        opool = ctx.enter_context(tc.tile_pool(name="opool", bufs=2))
        xt_r = xt_d.rearrange("(g p) m -> p g m", p=128)

        def x_load(t):
            msl = slice(t * MC, (t + 1) * MC)
            xsb = xpool.tile([128, NG, MC], BF16, tag="x")
            nc.sync.dma_start(out=xsb, in_=xt_r[:, :, msl])
            return xsb

        xq = []

        # software pipeline: issue gpair gp+1's front matmuls before gpair
        # gp's staircase so the in-order PE queue never stalls on the chain
        tiles = b_front(0)
        prev = None
        for gp in range(NGP):
            if prev is not None:
                b_chain(gp - 1, prev)
            prev_tiles = tiles
            if gp + 1 < NGP:
                tiles = b_front(gp + 1)
            prev = prev_tiles
            if gp == NGP - 2:
                # prefetch the first x chunks once phase B DMA traffic wanes
                xq = [x_load(0), x_load(1)]
        b_chain(NGP - 1, prev)

        # ---- phase C: stream x^T chunks, accumulate out tiles ---------------
        for t in range(NMC):
            msl = slice(t * MC, (t + 1) * MC)
            xsb = xq.pop(0)
            o_sb = opool.tile([128, MC // 128, OS], F32, tag="o")
            for j in range(MC // 128):
                p_out = psumX.tile([128, OS], F32, tag="ps")
                for g in range(NG):
                    nc.tensor.matmul(
                        p_out,
                        lhsT=xsb[:, g, j * 128 : (j + 1) * 128],
                        rhs=weff[:, g, :],
                        start=(g == 0),
                        stop=(g == NG - 1),
                    )
                nc.scalar.copy(o_sb[:, j, :], p_out)
            nc.sync.dma_start(
                out=out_d[msl, :].rearrange("(j p) o -> p j o", p=128),
                in_=o_sb,
            )
            if t + 2 < NMC:
                xq.append(x_load(t + 2))

    nc.compile()
    return nc


def _get_program(cb0, tk, dk, reps=1):
    key = (round(float(cb0), 9), tuple(round(float(t), 9) for t in tk),
           tuple(round(float(d), 9) for d in dk), reps)
    if key not in _cache:
        _cache[key] = _build_program(cb0, tk, dk, reps)
    return _cache[key]


def _make_in_maps(x, weight, scale_log, codebook, lora_A, lora_B):
    cb = np.asarray(codebook, dtype=np.float64)
    cb = cb / max(float(np.max(np.abs(cb))), 1e-8)
    dk = np.diff(cb)
    cm = float(cb[0]) + float(np.sum(dk)) * 0.5

    xb = np.ascontiguousarray(x.reshape(M, I), dtype=np.float32).astype(BF16_NP)
    xt = np.ascontiguousarray(xb.T)

    sl64 = np.exp(np.asarray(scale_log, dtype=np.float64).reshape(O, NG))
    s_full = sl64.astype(np.float32)            # S, f32 (sub-ulp host exp)
    is_full = (1.0 / sl64).astype(np.float32)   # 1/S, f32

    # indicator rows for the cm*S augmentation
    ind = np.zeros((NG, I), dtype=np.float32)
    for g in range(NG):
        ind[g, g * GROUP : (g + 1) * GROUP] = 1.0
    la_aug = np.concatenate(
        [np.ascontiguousarray(lora_A, dtype=np.float32), ind], axis=0)

    wt_full = np.ascontiguousarray(weight.T, dtype=np.float32)  # [I, O]

    in_maps = []
    for c in range(N_CORES):
        sl = slice(c * OS, (c + 1) * OS)
        lbt = lora_B[sl].T.astype(np.float32) * ALPHA_OVER_RANK  # [64, OS]
        lbt_aug = np.concatenate([lbt, cm * s_full[sl].T], axis=0)  # [96, OS]
        wt = wt_full[:, sl]                                    # [I, OS]
        wht = np.ascontiguousarray(wt).astype(BF16_NP)
        wlt = (wt - wht.astype(np.float32)).astype(BF16_NP)
        # S^T / invS^T replicated along i within each group: [I, OS]
        srt = np.repeat(s_full[sl].T.astype(BF16_NP), GROUP, axis=0)
        ist = np.repeat(is_full[sl].T, GROUP, axis=0)
        in_maps.append({
            "xt": xt,
            "wht": wht,
            "wlt": np.ascontiguousarray(wlt),
            "ist": np.ascontiguousarray(ist),
            "srt": np.ascontiguousarray(srt),
            "la": la_aug,
            "lbt": np.ascontiguousarray(lbt_aug),
        })
    return in_maps


def kernel(x, weight, scale_log, codebook, lora_A, lora_B):
    cb = np.asarray(codebook, dtype=np.float64)
    cb = cb / max(float(np.max(np.abs(cb))), 1e-8)
    tk = (cb[:-1] + cb[1:]) * 0.5
    dk = np.diff(cb)

    nc = _get_program(float(cb[0]), [float(v) for v in tk], [float(v) for v in dk])
    in_maps = _make_in_maps(x, weight, scale_log, codebook, lora_A, lora_B)

    res = run_bass_kernel_spmd(nc, in_maps, core_ids=list(range(N_CORES))).results
    out = np.concatenate([res[c]["out"] for c in range(N_CORES)], axis=1)
    return out.reshape(x.shape[0], x.shape[1], O)
